# Initial kernel scaffold
#
"""ChildSum TreeLSTM encoder kernel for Trainium2 (8 NeuronCores, SPMD).

Strategy: shard nodes (N) and edges (E) jointly by contiguous segment
ranges across 8 cores (segment_ids are sorted).  Within a core, nodes are
processed in 64 tiles of 128; each tile's child edges are padded to a
uniform number of 128-edge chunks (cmax) so all cores run one program.

Per 128-node tile t:
  fxb   = x @ W_f + b_f                       (PE, xT tiles as lhsT)
  per edge chunk: onehot[e,n] = (seg[e]==n)   (DVE is_equal vs iota)
      f_pre = onehotT.T @ fxb + prev_h @ U_f  (PE, PSUM accumulate)
      f = sigmoid(f_pre)                      (ACT)
      fc = f * prev_c                         (DVE)
      acc += onehot.T @ [prev_h | fc]         (PE scatter, PSUM accumulate)
  h_tilde = acc[:, :256]; fc_sum = acc[:, 256:]
  z = [x | h_tilde] @ W_combined + b_combined (PE, h_tildeT via PE transpose)
  c = sig(z_i)*tanh(z_u) + fc_sum ; h = sig(z_o)*tanh(c)
"""

import numpy as np

N, E, EDIM, HDIM = 65536, 262144, 300, 256
NC = 8
NLOC = N // NC          # 8192 nodes per core
P = 128
NT = NLOC // P          # 64 node tiles per core
KX = 3                  # xT K-chunks (384 = 300 + ones-row + pad)
XPAD = KX * P           # 384
KH = HDIM // P          # 2
KALL = KX + KH          # 5 K-chunks for W_combined


def _preprocess(x, prev_c, prev_h, W_combined, b_combined, W_f, U_f, b_f,
                segment_ids):
    seg = np.asarray(segment_ids).astype(np.int64)
    x = np.asarray(x, dtype=np.float32)
    prev_c = np.asarray(prev_c, dtype=np.float32)
    prev_h = np.asarray(prev_h, dtype=np.float32)
    W_combined = np.asarray(W_combined, dtype=np.float32)
    b_combined = np.asarray(b_combined, dtype=np.float32)
    W_f = np.asarray(W_f, dtype=np.float32)
    U_f = np.asarray(U_f, dtype=np.float32)
    b_f = np.asarray(b_f, dtype=np.float32)

    GT = N // P                          # 512 global node tiles
    starts = np.searchsorted(seg, np.arange(0, N + 1, P))
    cnt = np.diff(starts)                # edges per node tile
    cmax = int(np.ceil(cnt.max() / P))
    epc = cmax * P

    ar = np.arange(epc)
    idx = starts[:-1, None] + ar[None, :]          # [GT, epc]
    valid = ar[None, :] < cnt[:, None]             # [GT, epc]
    idxc = np.where(valid, np.minimum(idx, E - 1), 0)

    import ml_dtypes
    bf16 = ml_dtypes.bfloat16
    vf = valid.astype(np.float32)[:, :, None]
    ph = prev_h[idxc] * vf                         # [GT, epc, 256]
    pc = (prev_c[idxc] * vf).astype(bf16)
    pht = np.ascontiguousarray(ph.transpose(0, 2, 1)).astype(bf16)
    segrel = np.where(valid, seg[idxc] - P * np.arange(GT)[:, None],
                      -1).astype(np.float32)       # [GT, epc]

    # padded weights; ones-row folds biases into the matmuls
    wf_pad = np.zeros((XPAD, HDIM), np.float32)
    wf_pad[:EDIM] = W_f
    wf_pad[EDIM] = b_f
    wc_pad = np.zeros((XPAD + HDIM, 3 * HDIM), np.float32)
    wc_pad[:EDIM] = W_combined[:EDIM]
    wc_pad[EDIM] = b_combined
    wc_pad[XPAD:] = W_combined[EDIM:]

    in_maps = []
    for c in range(NC):
        xt = np.zeros((XPAD, NLOC), np.float32)
        xt[:EDIM] = x[c * NLOC:(c + 1) * NLOC].T
        xt[EDIM] = 1.0
        g0, g1 = c * NT, (c + 1) * NT
        in_maps.append({
            "xt": np.ascontiguousarray(xt),
            "wf": wf_pad,
            "wc": wc_pad,
            "uf": U_f.astype(bf16),
            "ph": np.ascontiguousarray(ph[g0:g1]),
            "pht": np.ascontiguousarray(pht[g0:g1]),
            "pc": np.ascontiguousarray(pc[g0:g1]),
            "seg": np.ascontiguousarray(segrel[g0:g1]),
        })
    return in_maps, cmax


def _build(cmax, nt=NT):
    import concourse.bass as bass
    import concourse.mybir as mybir
    import concourse.tile as tile
    from concourse import bacc
    from concourse.masks import make_identity

    dt = mybir.dt.float32
    bt = mybir.dt.bfloat16
    epc = cmax * P

    nc = bacc.Bacc("TRN2", target_bir_lowering=False, debug=False,
                   num_devices=NC)
    xt_d = nc.declare_dram_parameter("xt", [XPAD, NLOC], dt, isOutput=False)
    wf_d = nc.declare_dram_parameter("wf", [XPAD, HDIM], dt, isOutput=False)
    wc_d = nc.declare_dram_parameter("wc", [XPAD + HDIM, 3 * HDIM], dt,
                                     isOutput=False)
    uf_d = nc.declare_dram_parameter("uf", [HDIM, HDIM], bt, isOutput=False)
    ph_d = nc.declare_dram_parameter("ph", [NT, epc, HDIM], dt,
                                     isOutput=False)
    pht_d = nc.declare_dram_parameter("pht", [NT, HDIM, epc], bt,
                                      isOutput=False)
    pc_d = nc.declare_dram_parameter("pc", [NT, epc, HDIM], bt,
                                     isOutput=False)
    seg_d = nc.declare_dram_parameter("seg", [NT, epc], dt, isOutput=False)
    c_d = nc.declare_dram_parameter("c_out", [NLOC, HDIM], dt, isOutput=True)
    h_d = nc.declare_dram_parameter("h_out", [NLOC, HDIM], dt, isOutput=True)

    with tile.TileContext(nc) as tc:
        with (
            tc.tile_pool(name="const", bufs=1) as cpool,
            tc.tile_pool(name="inp", bufs=3) as ipool,
            tc.tile_pool(name="work", bufs=4) as wpool,
            tc.tile_pool(name="outp", bufs=3) as opool,
            tc.tile_pool(name="p_acc", bufs=2, space="PSUM") as p_acc,
            tc.tile_pool(name="p_fpre", bufs=2, space="PSUM") as p_fpre,
            tc.tile_pool(name="p_z", bufs=1, space="PSUM") as p_z,
            tc.tile_pool(name="p_small", bufs=1, space="PSUM") as p_small,
            tc.tile_pool(name="p_fxb", bufs=1, space="PSUM") as p_fxb,
        ):
            # constants
            wf_sb = cpool.tile([P, KX, HDIM], dt)
            nc.sync.dma_start(out=wf_sb[:],
                              in_=wf_d.ap().rearrange("(k p) n -> p k n", p=P))
            wc_sb = cpool.tile([P, KALL, 3 * HDIM], dt)
            nc.sync.dma_start(out=wc_sb[:],
                              in_=wc_d.ap().rearrange("(k p) n -> p k n", p=P))
            uf_sb = cpool.tile([P, KH, HDIM], bt)
            nc.sync.dma_start(out=uf_sb[:],
                              in_=uf_d.ap().rearrange("(k p) n -> p k n", p=P))
            iota_i = cpool.tile([P, P], mybir.dt.int32)
            nc.gpsimd.iota(iota_i[:], pattern=[[1, P]], base=0,
                           channel_multiplier=0)
            iota_row = cpool.tile([P, P], dt)
            nc.vector.tensor_copy(iota_row[:], iota_i[:])
            iota_ci = cpool.tile([P, 1], mybir.dt.int32)
            nc.gpsimd.iota(iota_ci[:], pattern=[[1, 1]], base=0,
                           channel_multiplier=1)
            iota_col = cpool.tile([P, 1], dt)
            nc.vector.tensor_copy(iota_col[:], iota_ci[:])
            ones_row = cpool.tile([1, P], dt)
            nc.gpsimd.memset(ones_row[:], 1.0)
            ident = cpool.tile([P, P], dt)
            make_identity(nc, ident[:])

            for t in range(nt):
                n0 = t * P
                # ---- loads ----
                xt = ipool.tile([P, KX, P], dt)
                nc.sync.dma_start(
                    out=xt[:],
                    in_=xt_d.ap().rearrange("(k p) n -> p k n", p=P)
                    [:, :, n0:n0 + P])
                rhs = ipool.tile([P, cmax, 2 * HDIM], dt, tag="rhs")
                nc.sync.dma_start(
                    out=rhs[:, :, 0:HDIM],
                    in_=ph_d.ap()[t].rearrange("(s p) h -> p s h", p=P))
                pht = ipool.tile([P, KH, epc], bt)
                nc.sync.dma_start(
                    out=pht[:],
                    in_=pht_d.ap()[t].rearrange("(k p) e -> p k e", p=P))
                pc = ipool.tile([P, cmax, HDIM], bt)
                nc.sync.dma_start(
                    out=pc[:],
                    in_=pc_d.ap()[t].rearrange("(s p) h -> p s h", p=P))
                segc = ipool.tile([P, cmax], dt)
                nc.sync.dma_start(
                    out=segc[:],
                    in_=seg_d.ap()[t].rearrange("(s p) -> p s", p=P))
                segr = ipool.tile([1, epc], dt)
                nc.sync.dma_start(out=segr[:], in_=seg_d.ap()[t:t + 1, :])

                # ---- fxb = x @ W_f + b_f for this node tile ----
                fxb_ps = p_fxb.tile([P, HDIM], dt)
                for k in range(KX):
                    nc.tensor.matmul(fxb_ps[:], lhsT=xt[:, k, :],
                                     rhs=wf_sb[:, k, :],
                                     start=(k == 0), stop=(k == KX - 1))
                fxb = wpool.tile([P, HDIM], dt)
                nc.vector.tensor_copy(fxb[:], fxb_ps[:])

                # ---- edge chunks: fpre phase (PE decoupled from ACT/DVE) ----
                rep0 = p_small.tile([P, P], dt, tag="small")
                nc.tensor.matmul(rep0[:], lhsT=ones_row[:],
                                 rhs=segr[:, 0:P], start=True, stop=True)
                reps = [rep0]
                for s in range(cmax):
                    onehotT = wpool.tile([P, P], dt)
                    nc.vector.tensor_tensor(
                        onehotT[:], iota_col[:].to_broadcast([P, P]),
                        reps[s][:], op=mybir.AluOpType.is_equal)
                    if s + 1 < cmax:
                        rep = p_small.tile([P, P], dt, tag="small")
                        nc.tensor.matmul(
                            rep[:], lhsT=ones_row[:],
                            rhs=segr[:, (s + 1) * P:(s + 2) * P],
                            start=True, stop=True)
                        reps.append(rep)
                    fpre = p_fpre.tile([P, HDIM], dt)
                    nc.tensor.matmul(fpre[:], lhsT=onehotT[:], rhs=fxb[:],
                                     start=True, stop=False)
                    for k in range(KH):
                        nc.tensor.matmul(fpre[:],
                                         lhsT=pht[:, k, s * P:(s + 1) * P],
                                         rhs=uf_sb[:, k, :],
                                         start=False, stop=(k == KH - 1))
                    f_sb = wpool.tile([P, HDIM], bt)
                    nc.scalar.activation(f_sb[:], fpre[:],
                                         mybir.ActivationFunctionType.Sigmoid)
                    nc.gpsimd.tensor_mul(rhs[:, s, HDIM:2 * HDIM], f_sb[:],
                                         pc[:, s, :])

                # ---- scatter phase ----
                acc = p_acc.tile([P, 2 * HDIM], dt)
                for s in range(cmax):
                    onehot = wpool.tile([P, P], dt)
                    nc.vector.tensor_tensor(
                        onehot[:], segc[:, s:s + 1].to_broadcast([P, P]),
                        iota_row[:], op=mybir.AluOpType.is_equal)
                    nc.tensor.matmul(acc[:], lhsT=onehot[:], rhs=rhs[:, s, :],
                                     start=(s == 0), stop=(s == cmax - 1))

                # ---- h_tildeT ----
                htld = wpool.tile([P, HDIM], dt)
                nc.vector.tensor_copy(htld[:], acc[:, 0:HDIM])
                htT = wpool.tile([P, KH, P], dt)
                for k in range(KH):
                    trp = p_small.tile([P, P], dt, tag="small")
                    nc.tensor.transpose(trp[:], htld[:, k * P:(k + 1) * P],
                                        ident[:])
                    nc.vector.tensor_copy(htT[:, k, :], trp[:])

                # ---- z = [x | h_tilde] @ W_combined + b_combined ----
                z = p_z.tile([P, 3 * HDIM], dt)
                for j in range(3):
                    for k in range(KALL):
                        lhsT = xt[:, k, :] if k < KX else htT[:, k - KX, :]
                        nc.tensor.matmul(
                            z[:, j * HDIM:(j + 1) * HDIM], lhsT=lhsT,
                            rhs=wc_sb[:, k, j * HDIM:(j + 1) * HDIM],
                            start=(k == 0), stop=(k == KALL - 1))

                # ---- gates ----
                szio = wpool.tile([P, 2 * HDIM], dt)
                nc.scalar.activation(szio[:], z[:, 0:2 * HDIM],
                                     mybir.ActivationFunctionType.Sigmoid)
                tzu = wpool.tile([P, HDIM], dt)
                nc.scalar.activation(tzu[:], z[:, 2 * HDIM:3 * HDIM],
                                     mybir.ActivationFunctionType.Tanh)
                ci = wpool.tile([P, HDIM], dt)
                nc.vector.tensor_mul(ci[:], szio[:, 0:HDIM], tzu[:])
                c_sb = opool.tile([P, HDIM], dt)
                nc.vector.tensor_add(c_sb[:], ci[:], acc[:, HDIM:2 * HDIM])
                tc_sb = wpool.tile([P, HDIM], dt)
                nc.scalar.activation(tc_sb[:], c_sb[:],
                                     mybir.ActivationFunctionType.Tanh)
                h_sb = opool.tile([P, HDIM], dt)
                nc.vector.tensor_mul(h_sb[:], szio[:, HDIM:2 * HDIM],
                                     tc_sb[:])
                nc.sync.dma_start(out=c_d.ap()[n0:n0 + P, :], in_=c_sb[:])
                nc.sync.dma_start(out=h_d.ap()[n0:n0 + P, :], in_=h_sb[:])

    nc.compile()
    return nc


def kernel(x, prev_c, prev_h, W_combined, b_combined, W_f, U_f, b_f,
           segment_ids, _trace=False):
    from concourse.bass_utils import run_bass_kernel_spmd

    in_maps, cmax = _preprocess(x, prev_c, prev_h, W_combined, b_combined,
                                W_f, U_f, b_f, segment_ids)
    nc = _build(cmax)
    res = run_bass_kernel_spmd(nc, in_maps, list(range(NC)), trace=_trace)
    c = np.concatenate([res.results[i]["c_out"] for i in range(NC)], axis=0)
    h = np.concatenate([res.results[i]["h_out"] for i in range(NC)], axis=0)
    kernel._last_exec_time_ns = res.exec_time_ns
    return (c, h)



# revision 3
# speedup vs baseline: 1.3658x; 1.3658x over previous
"""ChildSum TreeLSTM encoder kernel for Trainium2 (8 NeuronCores, SPMD).

v3 (on top of v2's all-bf16 matmuls):
 - one packed DRAM tensor per core -> 2 big input DMAs + 1 output DMA per
   128-node tile (was 8 small ones)
 - seg ids shipped replicated across partitions -> onehotT built by DVE
   directly, no PE outer-product / PSUM round-trip
 - fc multiply on DVE (fast, on critical path), onehot build on GPSIMD
 - manual PSUM bank layout with in-bank slot rotation: z io/u and fxb
   double-buffered across tiles
 - z matmuls fused to n=512 for the i/o gates

Per 128-node tile t:
  fxb   = x @ W_f + b_f                       (PE)
  z_x   = x-part of [x | h_tilde] @ W_combined (PE, early)
  per edge chunk: onehotT[n,e] = (seg[e]==n)  (DVE is_equal vs iota)
      f_pre = onehotT.T @ fxb + prev_h @ U_f  (PE, PSUM 2-slot)
      f = sigmoid(f_pre)                      (ACT)
      fc = f * prev_c                         (DVE)
      onehot[e,n] = (seg[e]==n)               (GPSIMD)
      acc += onehot.T @ [prev_h | fc]         (PE scatter, PSUM accumulate)
  h_tildeT via PE transpose; z_h tail matmuls
  c = sig(z_i)*tanh(z_u) + fc_sum ; h = sig(z_o)*tanh(c)
"""

import numpy as np

N, E, EDIM, HDIM = 65536, 262144, 300, 256
NC = 8
NLOC = N // NC          # 8192 nodes per core
P = 128
NT = NLOC // P          # 64 node tiles per core
KX = 3                  # xT K-chunks (384 = 300 + ones-row + pad)
XPAD = KX * P           # 384
KH = HDIM // P          # 2
KALL = KX + KH          # 5 K-chunks for W_combined


def _preprocess(x, prev_c, prev_h, W_combined, b_combined, W_f, U_f, b_f,
                segment_ids):
    import ml_dtypes
    bf16 = ml_dtypes.bfloat16

    seg = np.asarray(segment_ids).astype(np.int64)
    x = np.asarray(x, dtype=np.float32)
    prev_c = np.asarray(prev_c, dtype=np.float32)
    prev_h = np.asarray(prev_h, dtype=np.float32)
    W_combined = np.asarray(W_combined, dtype=np.float32)
    b_combined = np.asarray(b_combined, dtype=np.float32)
    W_f = np.asarray(W_f, dtype=np.float32)
    U_f = np.asarray(U_f, dtype=np.float32)
    b_f = np.asarray(b_f, dtype=np.float32)

    GT = N // P                          # 512 global node tiles
    starts = np.searchsorted(seg, np.arange(0, N + 1, P))
    cnt = np.diff(starts)                # edges per node tile
    cmax = int(np.ceil(cnt.max() / P))
    epc = cmax * P

    ar = np.arange(epc)
    idx = starts[:-1, None] + ar[None, :]          # [GT, epc]
    valid = ar[None, :] < cnt[:, None]             # [GT, epc]
    idxc = np.where(valid, np.minimum(idx, E - 1), 0)

    vf = valid.astype(np.float32)[:, :, None]
    ph = prev_h[idxc] * vf                         # [GT, epc, 256] f32
    pc = (prev_c[idxc] * vf).astype(bf16)          # [GT, epc, 256]
    phb = ph.astype(bf16)

    # edge-major, pre-chunked: [GT, P, cmax*H]  (partition = edge-in-chunk)
    ph_p = np.ascontiguousarray(
        phb.reshape(GT, cmax, P, HDIM).transpose(0, 2, 1, 3)
    ).reshape(GT, P, cmax * HDIM)
    pc_p = np.ascontiguousarray(
        pc.reshape(GT, cmax, P, HDIM).transpose(0, 2, 1, 3)
    ).reshape(GT, P, cmax * HDIM)
    # h-major (transposed), pre-chunked: [GT, P, KH*epc] (partition = h%128)
    pht = np.ascontiguousarray(phb.transpose(0, 2, 1))      # [GT, 256, epc]
    pht_p = np.ascontiguousarray(
        pht.reshape(GT, KH, P, epc).transpose(0, 2, 1, 3)
    ).reshape(GT, P, KH * epc)

    segrel = np.where(valid, seg[idxc] - P * np.arange(GT)[:, None],
                      -1).astype(np.int8)          # [GT, epc]
    segb = np.broadcast_to(segrel[:, None, :], (GT, P, epc))
    segc_p = np.full((GT, P, 8), -1, np.int8)
    segc_p[:, :, :cmax] = segrel.reshape(GT, cmax, P).transpose(0, 2, 1)

    # xT global, pre-chunked: [GT, P, KX*P]  (partition = x-row % 128)
    xt_g = np.zeros((XPAD, N), np.float32)
    xt_g[:EDIM] = x.T
    xt_g[EDIM] = 1.0
    xt_p = np.ascontiguousarray(
        xt_g.astype(bf16).reshape(KX, P, GT, P).transpose(2, 1, 0, 3)
    ).reshape(GT, P, KX * P)

    pk = np.concatenate([ph_p, pc_p, pht_p, xt_p], axis=2)  # bf16
    pk8i = np.concatenate([segb, segc_p], axis=2)    # [GT, P, 648] int8

    # padded weights; ones-row folds biases into the matmuls
    wf_pad = np.zeros((XPAD, HDIM), np.float32)
    wf_pad[:EDIM] = W_f
    wf_pad[EDIM] = b_f
    wc_pad = np.zeros((XPAD + HDIM, 3 * HDIM), np.float32)
    wc_pad[:EDIM] = W_combined[:EDIM]
    wc_pad[EDIM] = b_combined
    wc_pad[XPAD:] = W_combined[EDIM:]

    in_maps = []
    for c in range(NC):
        g0, g1 = c * NT, (c + 1) * NT
        in_maps.append({
            "pk": np.ascontiguousarray(pk[g0:g1]),
            "pk8i": np.ascontiguousarray(pk8i[g0:g1]),
            "wf": wf_pad.astype(bf16),
            "wc": wc_pad.astype(bf16),
            "uf": U_f.astype(bf16),
        })
    return in_maps, cmax


def _build(cmax, nt=NT, debug=None):
    import concourse.bass as bass
    import concourse.mybir as mybir
    import concourse.tile as tile
    from concourse import bacc
    from concourse.masks import make_identity

    dt = mybir.dt.float32
    bt = mybir.dt.bfloat16
    i8 = mybir.dt.int8
    epc = cmax * P
    H = HDIM
    # packed layout offsets (in elements) within pk[t, p, :]
    o_pc = cmax * H
    o_pht = o_pc + cmax * H
    o_xt = o_pht + KH * epc
    W = o_xt + KX * P

    nc = bacc.Bacc("TRN2", target_bir_lowering=False, debug=False,
                   num_devices=NC)
    pk_d = nc.declare_dram_parameter("pk", [NT, P, W], bt, isOutput=False)
    pk8i_d = nc.declare_dram_parameter("pk8i", [NT, P, epc + 8], i8,
                                       isOutput=False)
    wf_d = nc.declare_dram_parameter("wf", [XPAD, HDIM], bt, isOutput=False)
    wc_d = nc.declare_dram_parameter("wc", [XPAD + HDIM, 3 * HDIM], bt,
                                     isOutput=False)
    uf_d = nc.declare_dram_parameter("uf", [HDIM, HDIM], bt, isOutput=False)
    o_d = nc.declare_dram_parameter("out", [NT, P, 2 * HDIM], bt,
                                    isOutput=True)

    with tile.TileContext(nc) as tc:
        with (
            tc.tile_pool(name="const", bufs=1) as cpool,
            tc.tile_pool(name="inp", bufs=3) as ipool,
            tc.tile_pool(name="work", bufs=4) as wpool,
            tc.tile_pool(name="outp", bufs=3) as opool,
            tc.tile_pool(name="p_acc", bufs=1, space="PSUM") as p_acc,
            tc.tile_pool(name="p_fpre", bufs=1, space="PSUM") as p_fpre,
            tc.tile_pool(name="p_zio", bufs=2, space="PSUM") as p_zio,
            tc.tile_pool(name="p_fxb", bufs=1, space="PSUM") as p_fxb,
        ):
            # constants
            wf_sb = cpool.tile([P, KX, HDIM], bt)
            nc.sync.dma_start(out=wf_sb[:],
                              in_=wf_d.ap().rearrange("(k p) n -> p k n", p=P))
            wc_sb = cpool.tile([P, KALL, 3 * HDIM], bt)
            nc.sync.dma_start(out=wc_sb[:],
                              in_=wc_d.ap().rearrange("(k p) n -> p k n", p=P))
            uf_sb = cpool.tile([P, KH, HDIM], bt)
            nc.sync.dma_start(out=uf_sb[:],
                              in_=uf_d.ap().rearrange("(k p) n -> p k n", p=P))
            iota_i = cpool.tile([P, P], mybir.dt.int32)
            nc.gpsimd.iota(iota_i[:], pattern=[[1, P]], base=0,
                           channel_multiplier=0)
            iota_row5 = cpool.tile([P, cmax, P], i8)
            for s in range(cmax):
                nc.vector.tensor_copy(iota_row5[:, s, :], iota_i[:])
            iota_ci = cpool.tile([P, 1], mybir.dt.int32)
            nc.gpsimd.iota(iota_ci[:], pattern=[[1, 1]], base=0,
                           channel_multiplier=1)
            iota_col = cpool.tile([P, 1], i8)
            nc.vector.tensor_copy(iota_col[:], iota_ci[:])
            ident = cpool.tile([P, P], dt)
            make_identity(nc, ident[:])

            # persistent PSUM tiles (manual in-bank slot rotation)
            fpre6 = p_fpre.tile([P, 6, HDIM], dt, tag="fpre", bufs=1)
            zio2 = [p_zio.tile([P, 2 * HDIM], dt, name=f"zio{i}",
                               tag=f"zio{i}", bufs=1) for i in range(2)]
            # fxb group closes before the transposes run, so they can share
            # a bank; an open accumulation must never share a bank with a
            # start=True writer (the start clears the bank's has_written).
            fxtr = p_fxb.tile([P, 2 * HDIM], dt, tag="fxtr", bufs=1)
            fxb1 = fxtr[:, 0:HDIM]
            trp2 = [fxtr[:, HDIM + k * P:HDIM + (k + 1) * P]
                    for k in range(KH)]
            # acc double-buffers; zu(t) borrows the *other* acc buffer's
            # first half (its scatter won't start until well after tzu reads)
            acc2 = [p_acc.tile([P, 2 * HDIM], dt, name=f"acc{i}",
                               tag=f"acc{i}", bufs=1) for i in range(2)]

            def front(t):
                """loads + onehots + fxb + zio-x + fpre/sigmoid/fc of tile t"""
                rhs = ipool.tile([P, cmax, 2 * HDIM], bt, tag="rhs", bufs=4)
                nc.sync.dma_start(
                    out=rhs[:, :, 0:HDIM],
                    in_=pk_d.ap()[t][:, 0:o_pc].rearrange(
                        "p (s h) -> p s h", s=cmax))
                big = ipool.tile([P, W - o_pc], bt, tag="big", bufs=4)
                nc.sync.dma_start(out=big[:], in_=pk_d.ap()[t][:, o_pc:W])
                seg8 = ipool.tile([P, epc + 8], i8, tag="seg8", bufs=4)
                nc.sync.dma_start(out=seg8[:], in_=pk8i_d.ap()[t])

                def pcv(s, big=big):
                    return big[:, s * H:(s + 1) * H]

                def phtv(k, s, big=big):
                    b = o_pht - o_pc + k * epc + s * P
                    return big[:, b:b + P]

                def xtv(k, big=big):
                    b = o_xt - o_pc + k * P
                    return big[:, b:b + P]

                zio = zio2[t % 2]

                ohT_all = wpool.tile([P, epc], bt, tag="ohT", bufs=2)
                nc.vector.tensor_tensor(
                    ohT_all[:], iota_col[:].to_broadcast([P, epc]),
                    seg8[:, 0:epc], op=mybir.AluOpType.is_equal)
                oh_all = wpool.tile([P, cmax, P], bt, tag="oh", bufs=2)
                nc.vector.tensor_tensor(
                    oh_all[:], seg8[:, epc:epc + cmax]
                    .to_broadcast([P, cmax, P]),
                    iota_row5[:], op=mybir.AluOpType.is_equal)

                for k in range(KX):
                    nc.tensor.matmul(fxb1, lhsT=xtv(k), rhs=wf_sb[:, k, :],
                                     start=(k == 0), stop=(k == KX - 1))
                fxb = wpool.tile([P, HDIM], bt, tag="fxb", bufs=2)
                nc.vector.tensor_copy(fxb[:], fxb1)

                for k in range(KX):
                    nc.tensor.matmul(zio[:], lhsT=xtv(k),
                                     rhs=wc_sb[:, k, 0:2 * HDIM],
                                     start=(k == 0), stop=False)

                npair = (cmax + 1) // 2
                for g in range(npair):
                    s0 = 2 * g
                    ns = min(2, cmax - s0)
                    base = (g % 3) * 2
                    for i in range(ns):
                        s = s0 + i
                        fpre = fpre6[:, base + i, :]
                        nc.tensor.matmul(fpre,
                                         lhsT=ohT_all[:, s * P:(s + 1) * P],
                                         rhs=fxb[:], start=True, stop=False)
                        for k in range(KH):
                            nc.tensor.matmul(fpre, lhsT=phtv(k, s),
                                             rhs=uf_sb[:, k, :],
                                             start=False, stop=(k == KH - 1))
                    f_sb = wpool.tile([P, 2, HDIM], bt, tag="f", bufs=3)
                    nc.scalar.activation(
                        f_sb[:, 0:ns, :], fpre6[:, base:base + ns, :],
                        mybir.ActivationFunctionType.Sigmoid)
                    for i in range(ns):
                        s = s0 + i
                        nc.vector.tensor_mul(rhs[:, s, HDIM:2 * HDIM],
                                             f_sb[:, i, :], pcv(s))

                return dict(t=t, rhs=rhs, oh_all=oh_all, zio=zio,
                            xtv=xtv, fxb=fxb)

            def back(st):
                """scatter + h_tildeT + z tails + gates + out of tile t"""
                t = st["t"]
                rhs, oh_all, zio, xtv = (st["rhs"], st["oh_all"], st["zio"],
                                         st["xtv"])
                # bank roles alternate: h_acc(t) in bank t%2; fc_acc(t) and
                # zu(t) in bank (t+1)%2 (whose groups close before that
                # bank's next start=True clears it)
                hacc = acc2[t % 2][:, 0:HDIM]
                fcacc = acc2[(t + 1) % 2][:, HDIM:2 * HDIM]
                zu1 = acc2[(t + 1) % 2][:, 0:HDIM]

                for s in range(cmax):
                    nc.tensor.matmul(hacc, lhsT=oh_all[:, s, :],
                                     rhs=rhs[:, s, 0:HDIM],
                                     start=(s == 0), stop=(s == cmax - 1))
                htld = wpool.tile([P, HDIM], dt, tag="htld", bufs=2)
                nc.scalar.activation(htld[:], hacc,
                                     mybir.ActivationFunctionType.Copy)
                for s in range(cmax):
                    nc.tensor.matmul(fcacc, lhsT=oh_all[:, s, :],
                                     rhs=rhs[:, s, HDIM:2 * HDIM],
                                     start=(s == 0), stop=(s == cmax - 1))
                nc.tensor.transpose(trp2[0], htld[:, 0:P], ident[:])
                for k in range(KX):
                    nc.tensor.matmul(zu1, lhsT=xtv(k),
                                     rhs=wc_sb[:, k, 2 * HDIM:3 * HDIM],
                                     start=(k == 0), stop=False)
                nc.tensor.transpose(trp2[1], htld[:, P:2 * P], ident[:])
                htT = wpool.tile([P, KH, P], bt, tag="htT", bufs=2)
                for k in range(KH):
                    nc.vector.tensor_copy(htT[:, k, :], trp2[k])

                for k in range(KH):
                    kk = KX + k
                    nc.tensor.matmul(zio[:], lhsT=htT[:, k, :],
                                     rhs=wc_sb[:, kk, 0:2 * HDIM],
                                     start=False, stop=(k == KH - 1))
                    nc.tensor.matmul(zu1, lhsT=htT[:, k, :],
                                     rhs=wc_sb[:, kk, 2 * HDIM:3 * HDIM],
                                     start=False, stop=(k == KH - 1))

                if debug == "acc":
                    out_t = opool.tile([P, 2 * HDIM], bt)
                    nc.vector.tensor_copy(out_t[:, 0:HDIM], hacc)
                    nc.vector.tensor_copy(out_t[:, HDIM:2 * HDIM], fcacc)
                    nc.sync.dma_start(out=o_d.ap()[t], in_=out_t[:])
                    return
                if debug == "z":
                    out_t = opool.tile([P, 2 * HDIM], bt)
                    nc.vector.tensor_copy(out_t[:, 0:HDIM], zio[:, 0:HDIM])
                    nc.vector.tensor_copy(out_t[:, HDIM:2 * HDIM], zu1)
                    nc.sync.dma_start(out=o_d.ap()[t], in_=out_t[:])
                    return

                out_t = opool.tile([P, 2 * HDIM], bt)
                szio = wpool.tile([P, 2 * HDIM], dt, tag="szio", bufs=2)
                nc.scalar.activation(szio[:], zio[:],
                                     mybir.ActivationFunctionType.Sigmoid)
                tzu = wpool.tile([P, HDIM], dt, tag="tzu", bufs=2)
                nc.scalar.activation(tzu[:], zu1,
                                     mybir.ActivationFunctionType.Tanh)
                ci = wpool.tile([P, HDIM], dt, tag="ci", bufs=2)
                nc.gpsimd.tensor_mul(ci[:], szio[:, 0:HDIM], tzu[:])
                nc.vector.tensor_add(out_t[:, 0:HDIM], ci[:], fcacc)
                tc_sb = wpool.tile([P, HDIM], dt, tag="tc", bufs=2)
                nc.scalar.activation(tc_sb[:], out_t[:, 0:HDIM],
                                     mybir.ActivationFunctionType.Tanh)
                nc.gpsimd.tensor_mul(out_t[:, HDIM:2 * HDIM],
                                     szio[:, HDIM:2 * HDIM], tc_sb[:])
                nc.sync.dma_start(out=o_d.ap()[t], in_=out_t[:])

            pend = None
            for t in range(nt):
                st = front(t)
                if pend is not None:
                    back(pend)
                pend = st
            back(pend)

    nc.compile()
    return nc


def kernel(x, prev_c, prev_h, W_combined, b_combined, W_f, U_f, b_f,
           segment_ids, _trace=False, _debug=None):
    from concourse.bass_utils import run_bass_kernel_spmd

    in_maps, cmax = _preprocess(x, prev_c, prev_h, W_combined, b_combined,
                                W_f, U_f, b_f, segment_ids)
    nc = _build(cmax, debug=_debug)
    res = run_bass_kernel_spmd(nc, in_maps, list(range(NC)), trace=_trace)
    cs, hs = [], []
    for i in range(NC):
        o = np.asarray(res.results[i]["out"]).astype(np.float32)
        o = o.reshape(NLOC, 2 * HDIM)
        cs.append(o[:, :HDIM])
        hs.append(o[:, HDIM:])
    c = np.concatenate(cs, axis=0)
    h = np.concatenate(hs, axis=0)
    kernel._last_exec_time_ns = res.exec_time_ns
    return (c, h)


# revision 5
# speedup vs baseline: 1.4349x; 1.0506x over previous
"""ChildSum TreeLSTM encoder kernel for Trainium2 (8 NeuronCores, SPMD).

v3 (on top of v2's all-bf16 matmuls):
 - one packed DRAM tensor per core -> 2 big input DMAs + 1 output DMA per
   128-node tile (was 8 small ones)
 - seg ids shipped replicated across partitions -> onehotT built by DVE
   directly, no PE outer-product / PSUM round-trip
 - fc multiply on DVE (fast, on critical path), onehot build on GPSIMD
 - manual PSUM bank layout with in-bank slot rotation: z io/u and fxb
   double-buffered across tiles
 - z matmuls fused to n=512 for the i/o gates

Per 128-node tile t:
  fxb   = x @ W_f + b_f                       (PE)
  z_x   = x-part of [x | h_tilde] @ W_combined (PE, early)
  per edge chunk: onehotT[n,e] = (seg[e]==n)  (DVE is_equal vs iota)
      f_pre = onehotT.T @ fxb + prev_h @ U_f  (PE, PSUM 2-slot)
      f = sigmoid(f_pre)                      (ACT)
      fc = f * prev_c                         (DVE)
      onehot[e,n] = (seg[e]==n)               (GPSIMD)
      acc += onehot.T @ [prev_h | fc]         (PE scatter, PSUM accumulate)
  h_tildeT via PE transpose; z_h tail matmuls
  c = sig(z_i)*tanh(z_u) + fc_sum ; h = sig(z_o)*tanh(c)
"""

import numpy as np

N, E, EDIM, HDIM = 65536, 262144, 300, 256
NC = 8
NLOC = N // NC          # 8192 nodes per core
P = 128
NT = NLOC // P          # 64 node tiles per core
KX = 3                  # xT K-chunks (384 = 300 + ones-row + pad)
XPAD = KX * P           # 384
KH = HDIM // P          # 2
KALL = KX + KH          # 5 K-chunks for W_combined


def _preprocess(x, prev_c, prev_h, W_combined, b_combined, W_f, U_f, b_f,
                segment_ids):
    import ml_dtypes
    bf16 = ml_dtypes.bfloat16

    seg = np.asarray(segment_ids).astype(np.int64)
    x = np.asarray(x, dtype=np.float32)
    prev_c = np.asarray(prev_c, dtype=np.float32)
    prev_h = np.asarray(prev_h, dtype=np.float32)
    W_combined = np.asarray(W_combined, dtype=np.float32)
    b_combined = np.asarray(b_combined, dtype=np.float32)
    W_f = np.asarray(W_f, dtype=np.float32)
    U_f = np.asarray(U_f, dtype=np.float32)
    b_f = np.asarray(b_f, dtype=np.float32)

    GT = N // P                          # 512 global node tiles
    starts = np.searchsorted(seg, np.arange(0, N + 1, P))
    cnt = np.diff(starts)                # edges per node tile
    cmax = int(np.ceil(cnt.max() / P))
    epc = cmax * P

    ar = np.arange(epc)
    idx = starts[:-1, None] + ar[None, :]          # [GT, epc]
    valid = ar[None, :] < cnt[:, None]             # [GT, epc]
    idxc = np.where(valid, np.minimum(idx, E - 1), 0)

    vf = valid.astype(np.float32)[:, :, None]
    ph = prev_h[idxc] * vf                         # [GT, epc, 256] f32
    pc = (prev_c[idxc] * vf).astype(bf16)          # [GT, epc, 256]
    phb = ph.astype(bf16)

    # edge-major, pre-chunked: [GT, P, cmax*H]  (partition = edge-in-chunk)
    ph_p = np.ascontiguousarray(
        phb.reshape(GT, cmax, P, HDIM).transpose(0, 2, 1, 3)
    ).reshape(GT, P, cmax * HDIM)
    pc_p = np.ascontiguousarray(
        pc.reshape(GT, cmax, P, HDIM).transpose(0, 2, 1, 3)
    ).reshape(GT, P, cmax * HDIM)
    # h-major (transposed), pre-chunked: [GT, P, KH*epc] (partition = h%128)
    pht = np.ascontiguousarray(phb.transpose(0, 2, 1))      # [GT, 256, epc]
    pht_p = np.ascontiguousarray(
        pht.reshape(GT, KH, P, epc).transpose(0, 2, 1, 3)
    ).reshape(GT, P, KH * epc)

    segrel = np.where(valid, seg[idxc] - P * np.arange(GT)[:, None],
                      -1).astype(np.int8)          # [GT, epc]
    segb = np.broadcast_to(segrel[:, None, :], (GT, P, epc))
    segc_p = np.full((GT, P, 8), -1, np.int8)
    segc_p[:, :, :cmax] = segrel.reshape(GT, cmax, P).transpose(0, 2, 1)

    # xT global, pre-chunked: [GT, P, KX*P]  (partition = x-row % 128)
    xt_g = np.zeros((XPAD, N), np.float32)
    xt_g[:EDIM] = x.T
    xt_g[EDIM] = 1.0
    xt_p = np.ascontiguousarray(
        xt_g.astype(bf16).reshape(KX, P, GT, P).transpose(2, 1, 0, 3)
    ).reshape(GT, P, KX * P)

    pk = np.concatenate([ph_p, pc_p, pht_p, xt_p], axis=2)  # bf16
    pk8i = np.concatenate([segb, segc_p], axis=2)    # [GT, P, 648] int8

    # padded weights; ones-row folds biases into the matmuls
    wf_pad = np.zeros((XPAD, HDIM), np.float32)
    wf_pad[:EDIM] = W_f
    wf_pad[EDIM] = b_f
    wc_pad = np.zeros((XPAD + HDIM, 3 * HDIM), np.float32)
    wc_pad[:EDIM] = W_combined[:EDIM]
    wc_pad[EDIM] = b_combined
    wc_pad[XPAD:] = W_combined[EDIM:]

    in_maps = []
    for c in range(NC):
        g0, g1 = c * NT, (c + 1) * NT
        in_maps.append({
            "pk": np.ascontiguousarray(pk[g0:g1]),
            "pk8i": np.ascontiguousarray(pk8i[g0:g1]),
            "wf": wf_pad.astype(bf16),
            "wc": wc_pad.astype(bf16),
            "uf": U_f.astype(bf16),
        })
    return in_maps, cmax


def _build(cmax, nt=NT, debug=None):
    import concourse.bass as bass
    import concourse.mybir as mybir
    import concourse.tile as tile
    from concourse import bacc

    dt = mybir.dt.float32
    bt = mybir.dt.bfloat16
    i8 = mybir.dt.int8
    epc = cmax * P
    H = HDIM
    # packed layout offsets (in elements) within pk[t, p, :]
    o_pc = cmax * H
    o_pht = o_pc + cmax * H
    o_xt = o_pht + KH * epc
    W = o_xt + KX * P

    nc = bacc.Bacc("TRN2", target_bir_lowering=False, debug=False,
                   num_devices=NC)
    pk_d = nc.declare_dram_parameter("pk", [NT, P, W], bt, isOutput=False)
    pk8i_d = nc.declare_dram_parameter("pk8i", [NT, P, epc + 8], i8,
                                       isOutput=False)
    wf_d = nc.declare_dram_parameter("wf", [XPAD, HDIM], bt, isOutput=False)
    wc_d = nc.declare_dram_parameter("wc", [XPAD + HDIM, 3 * HDIM], bt,
                                     isOutput=False)
    uf_d = nc.declare_dram_parameter("uf", [HDIM, HDIM], bt, isOutput=False)
    o_d = nc.declare_dram_parameter("out", [NT, P, 2 * HDIM], bt,
                                    isOutput=True)

    with tile.TileContext(nc) as tc:
        with (
            tc.tile_pool(name="const", bufs=1) as cpool,
            tc.tile_pool(name="inp", bufs=3) as ipool,
            tc.tile_pool(name="work", bufs=4) as wpool,
            tc.tile_pool(name="outp", bufs=3) as opool,
            tc.tile_pool(name="p_acc", bufs=1, space="PSUM") as p_acc,
            tc.tile_pool(name="p_fpre", bufs=1, space="PSUM") as p_fpre,
            tc.tile_pool(name="p_zio", bufs=2, space="PSUM") as p_zio,
            tc.tile_pool(name="p_fxb", bufs=1, space="PSUM") as p_fxb,
        ):
            # constants
            wf_sb = cpool.tile([P, KX, HDIM], bt)
            nc.sync.dma_start(out=wf_sb[:],
                              in_=wf_d.ap().rearrange("(k p) n -> p k n", p=P))
            wc_sb = cpool.tile([P, KALL, 3 * HDIM], bt)
            nc.sync.dma_start(out=wc_sb[:],
                              in_=wc_d.ap().rearrange("(k p) n -> p k n", p=P))
            uf_sb = cpool.tile([P, KH, HDIM], bt)
            nc.sync.dma_start(out=uf_sb[:],
                              in_=uf_d.ap().rearrange("(k p) n -> p k n", p=P))
            iota_i = cpool.tile([P, P], mybir.dt.int32)
            nc.gpsimd.iota(iota_i[:], pattern=[[1, P]], base=0,
                           channel_multiplier=0)
            iota_row5 = cpool.tile([P, cmax, P], i8)
            for s in range(cmax):
                nc.vector.tensor_copy(iota_row5[:, s, :], iota_i[:])
            iota_ci = cpool.tile([P, 1], mybir.dt.int32)
            nc.gpsimd.iota(iota_ci[:], pattern=[[1, 1]], base=0,
                           channel_multiplier=1)
            iota_col = cpool.tile([P, 1], i8)
            nc.vector.tensor_copy(iota_col[:], iota_ci[:])

            # persistent PSUM tiles (manual in-bank slot rotation)
            fpre6 = p_fpre.tile([P, 6, HDIM], dt, tag="fpre", bufs=1)
            zio2 = [p_zio.tile([P, 2 * HDIM], dt, name=f"zio{i}",
                               tag=f"zio{i}", bufs=1) for i in range(2)]
            # fxb group closes before the transposes run, so they can share
            # a bank; an open accumulation must never share a bank with a
            # start=True writer (the start clears the bank's has_written).
            fxtr = p_fxb.tile([P, 2 * HDIM], dt, tag="fxtr", bufs=1)
            fxb1 = fxtr[:, 0:HDIM]
            # acc double-buffers; zu(t) borrows the *other* acc buffer's
            # first half (its scatter won't start until well after tzu reads)
            acc2 = [p_acc.tile([P, 2 * HDIM], dt, name=f"acc{i}",
                               tag=f"acc{i}", bufs=1) for i in range(2)]

            def front(t):
                """loads + onehots + fxb + zio-x + fpre/sigmoid/fc of tile t"""
                big = ipool.tile([P, W], bt, tag="big", bufs=4)
                nc.sync.dma_start(out=big[:], in_=pk_d.ap()[t])
                seg8 = ipool.tile([P, epc + 8], i8, tag="seg8", bufs=4)
                nc.sync.dma_start(out=seg8[:], in_=pk8i_d.ap()[t])

                def phv(s, k, big=big):
                    b = s * H + k * P
                    return big[:, b:b + P]

                def pcv(s, big=big):
                    return big[:, o_pc + s * H:o_pc + (s + 1) * H]

                def phtv(k, s, big=big):
                    b = o_pht + k * epc + s * P
                    return big[:, b:b + P]

                def xtv(k, big=big):
                    b = o_xt + k * P
                    return big[:, b:b + P]

                zio = zio2[t % 2]
                fc_t = wpool.tile([P, cmax, HDIM], bt, tag="fc", bufs=2)

                ohT_all = wpool.tile([P, epc], bt, tag="ohT", bufs=2)
                nc.vector.tensor_tensor(
                    ohT_all[:], iota_col[:].to_broadcast([P, epc]),
                    seg8[:, 0:epc], op=mybir.AluOpType.is_equal)
                oh_all = wpool.tile([P, cmax, P], bt, tag="oh", bufs=2)
                nc.vector.tensor_tensor(
                    oh_all[:], seg8[:, epc:epc + cmax]
                    .to_broadcast([P, cmax, P]),
                    iota_row5[:], op=mybir.AluOpType.is_equal)

                for k in range(KX):
                    nc.tensor.matmul(fxb1, lhsT=xtv(k), rhs=wf_sb[:, k, :],
                                     start=(k == 0), stop=(k == KX - 1))
                fxb = wpool.tile([P, HDIM], bt, tag="fxb", bufs=2)
                nc.any.tensor_copy(fxb[:], fxb1)

                for k in range(KX):
                    nc.tensor.matmul(zio[:], lhsT=xtv(k),
                                     rhs=wc_sb[:, k, 0:2 * HDIM],
                                     start=(k == 0), stop=False)

                npair = (cmax + 1) // 2
                for g in range(npair):
                    s0 = 2 * g
                    ns = min(2, cmax - s0)
                    base = (g % 3) * 2
                    for i in range(ns):
                        s = s0 + i
                        fpre = fpre6[:, base + i, :]
                        nc.tensor.matmul(fpre,
                                         lhsT=ohT_all[:, s * P:(s + 1) * P],
                                         rhs=fxb[:], start=True, stop=False)
                        for k in range(KH):
                            nc.tensor.matmul(fpre, lhsT=phtv(k, s),
                                             rhs=uf_sb[:, k, :],
                                             start=False, stop=(k == KH - 1))
                    f_sb = wpool.tile([P, 2, HDIM], bt, tag="f", bufs=3)
                    nc.scalar.activation(
                        f_sb[:, 0:ns, :], fpre6[:, base:base + ns, :],
                        mybir.ActivationFunctionType.Sigmoid)
                    for i in range(ns):
                        s = s0 + i
                        nc.any.tensor_mul(fc_t[:, s, :], f_sb[:, i, :],
                                          pcv(s))

                return dict(t=t, fc_t=fc_t, oh_all=oh_all, zio=zio,
                            xtv=xtv, phv=phv, fxb=fxb)

            def back(st):
                """scatter + h_tildeT + z tails + gates + out of tile t"""
                t = st["t"]
                fc_t, oh_all, zio, xtv, phv = (
                    st["fc_t"], st["oh_all"], st["zio"], st["xtv"], st["phv"])
                # bank roles alternate: h_tildeT(t) in bank t%2; fc_acc(t)
                # and zu(t) in bank (t+1)%2 (their groups close before that
                # bank's next start=True clears it)
                htTps = acc2[t % 2][:, 0:HDIM]
                fcacc = acc2[(t + 1) % 2][:, HDIM:2 * HDIM]
                zu1 = acc2[(t + 1) % 2][:, 0:HDIM]

                # transposed h-scatter: h_tildeT[h, n] = sum_e ph[e,h]oh[e,n]
                # (single start=True: the bank-level has_written clear covers
                # both k-regions)
                for s in range(cmax):
                    for k in range(KH):
                        nc.tensor.matmul(htTps[:, k * P:(k + 1) * P],
                                         lhsT=phv(s, k),
                                         rhs=oh_all[:, s, :],
                                         start=(s == 0 and k == 0),
                                         stop=(s == cmax - 1 and k == KH - 1))
                for s in range(cmax):
                    nc.tensor.matmul(fcacc, lhsT=oh_all[:, s, :],
                                     rhs=fc_t[:, s, :],
                                     start=(s == 0), stop=(s == cmax - 1))
                htT = wpool.tile([P, KH * P], bt, tag="htT", bufs=2)
                nc.any.tensor_copy(htT[:], htTps)
                for k in range(KX):
                    nc.tensor.matmul(zu1, lhsT=xtv(k),
                                     rhs=wc_sb[:, k, 2 * HDIM:3 * HDIM],
                                     start=(k == 0), stop=False)

                for k in range(KH):
                    kk = KX + k
                    nc.tensor.matmul(zio[:], lhsT=htT[:, k * P:(k + 1) * P],
                                     rhs=wc_sb[:, kk, 0:2 * HDIM],
                                     start=False, stop=(k == KH - 1))
                    nc.tensor.matmul(zu1, lhsT=htT[:, k * P:(k + 1) * P],
                                     rhs=wc_sb[:, kk, 2 * HDIM:3 * HDIM],
                                     start=False, stop=(k == KH - 1))

                if debug == "acc":
                    out_t = opool.tile([P, 2 * HDIM], bt)
                    nc.vector.tensor_copy(out_t[:, 0:HDIM], htTps)
                    nc.vector.tensor_copy(out_t[:, HDIM:2 * HDIM], fcacc)
                    nc.sync.dma_start(out=o_d.ap()[t], in_=out_t[:])
                    return
                if debug == "z":
                    out_t = opool.tile([P, 2 * HDIM], bt)
                    nc.vector.tensor_copy(out_t[:, 0:HDIM], zio[:, 0:HDIM])
                    nc.vector.tensor_copy(out_t[:, HDIM:2 * HDIM], zu1)
                    nc.sync.dma_start(out=o_d.ap()[t], in_=out_t[:])
                    return

                out_t = opool.tile([P, 2 * HDIM], bt)
                szio = wpool.tile([P, 2 * HDIM], dt, tag="szio", bufs=2)
                nc.scalar.activation(szio[:], zio[:],
                                     mybir.ActivationFunctionType.Sigmoid)
                tzu = wpool.tile([P, HDIM], dt, tag="tzu", bufs=2)
                nc.scalar.activation(tzu[:], zu1,
                                     mybir.ActivationFunctionType.Tanh)
                ci = wpool.tile([P, HDIM], dt, tag="ci", bufs=2)
                nc.any.tensor_mul(ci[:], szio[:, 0:HDIM], tzu[:])
                nc.any.tensor_add(out_t[:, 0:HDIM], ci[:], fcacc)
                tc_sb = wpool.tile([P, HDIM], dt, tag="tc", bufs=2)
                nc.scalar.activation(tc_sb[:], out_t[:, 0:HDIM],
                                     mybir.ActivationFunctionType.Tanh)
                nc.any.tensor_mul(out_t[:, HDIM:2 * HDIM],
                                  szio[:, HDIM:2 * HDIM], tc_sb[:])
                nc.sync.dma_start(out=o_d.ap()[t], in_=out_t[:])

            pend = None
            for t in range(nt):
                st = front(t)
                if pend is not None:
                    back(pend)
                pend = st
            back(pend)

    nc.compile()
    return nc


def kernel(x, prev_c, prev_h, W_combined, b_combined, W_f, U_f, b_f,
           segment_ids, _trace=False, _debug=None):
    from concourse.bass_utils import run_bass_kernel_spmd

    in_maps, cmax = _preprocess(x, prev_c, prev_h, W_combined, b_combined,
                                W_f, U_f, b_f, segment_ids)
    nc = _build(cmax, debug=_debug)
    res = run_bass_kernel_spmd(nc, in_maps, list(range(NC)), trace=_trace)
    cs, hs = [], []
    for i in range(NC):
        o = np.asarray(res.results[i]["out"]).astype(np.float32)
        o = o.reshape(NLOC, 2 * HDIM)
        cs.append(o[:, :HDIM])
        hs.append(o[:, HDIM:])
    c = np.concatenate(cs, axis=0)
    h = np.concatenate(hs, axis=0)
    kernel._last_exec_time_ns = res.exec_time_ns
    return (c, h)


# revision 6
# speedup vs baseline: 1.4513x; 1.0115x over previous
"""ChildSum TreeLSTM encoder kernel for Trainium2 (8 NeuronCores, SPMD).

v3 (on top of v2's all-bf16 matmuls):
 - one packed DRAM tensor per core -> 2 big input DMAs + 1 output DMA per
   128-node tile (was 8 small ones)
 - seg ids shipped replicated across partitions -> onehotT built by DVE
   directly, no PE outer-product / PSUM round-trip
 - fc multiply on DVE (fast, on critical path), onehot build on GPSIMD
 - manual PSUM bank layout with in-bank slot rotation: z io/u and fxb
   double-buffered across tiles
 - z matmuls fused to n=512 for the i/o gates

Per 128-node tile t:
  fxb   = x @ W_f + b_f                       (PE)
  z_x   = x-part of [x | h_tilde] @ W_combined (PE, early)
  per edge chunk: onehotT[n,e] = (seg[e]==n)  (DVE is_equal vs iota)
      f_pre = onehotT.T @ fxb + prev_h @ U_f  (PE, PSUM 2-slot)
      f = sigmoid(f_pre)                      (ACT)
      fc = f * prev_c                         (DVE)
      onehot[e,n] = (seg[e]==n)               (GPSIMD)
      acc += onehot.T @ [prev_h | fc]         (PE scatter, PSUM accumulate)
  h_tildeT via PE transpose; z_h tail matmuls
  c = sig(z_i)*tanh(z_u) + fc_sum ; h = sig(z_o)*tanh(c)
"""

import numpy as np

N, E, EDIM, HDIM = 65536, 262144, 300, 256
NC = 8
NLOC = N // NC          # 8192 nodes per core
P = 128
NT = NLOC // P          # 64 node tiles per core
KX = 3                  # xT K-chunks (384 = 300 + ones-row + pad)
XPAD = KX * P           # 384
KH = HDIM // P          # 2
KALL = KX + KH          # 5 K-chunks for W_combined


def _preprocess(x, prev_c, prev_h, W_combined, b_combined, W_f, U_f, b_f,
                segment_ids):
    import ml_dtypes
    bf16 = ml_dtypes.bfloat16

    seg = np.asarray(segment_ids).astype(np.int64)
    x = np.asarray(x, dtype=np.float32)
    prev_c = np.asarray(prev_c, dtype=np.float32)
    prev_h = np.asarray(prev_h, dtype=np.float32)
    W_combined = np.asarray(W_combined, dtype=np.float32)
    b_combined = np.asarray(b_combined, dtype=np.float32)
    W_f = np.asarray(W_f, dtype=np.float32)
    U_f = np.asarray(U_f, dtype=np.float32)
    b_f = np.asarray(b_f, dtype=np.float32)

    GT = N // P                          # 512 global node tiles
    starts = np.searchsorted(seg, np.arange(0, N + 1, P))
    cnt = np.diff(starts)                # edges per node tile
    cmax = int(np.ceil(cnt.max() / P))
    epc = cmax * P

    ar = np.arange(epc)
    idx = starts[:-1, None] + ar[None, :]          # [GT, epc]
    valid = ar[None, :] < cnt[:, None]             # [GT, epc]
    idxc = np.where(valid, np.minimum(idx, E - 1), 0)

    vf = valid.astype(np.float32)[:, :, None]
    ph = prev_h[idxc] * vf                         # [GT, epc, 256] f32
    pc = (prev_c[idxc] * vf).astype(bf16)          # [GT, epc, 256]
    phb = ph.astype(bf16)

    # edge-major, pre-chunked: [GT, P, cmax*H]  (partition = edge-in-chunk)
    ph_p = np.ascontiguousarray(
        phb.reshape(GT, cmax, P, HDIM).transpose(0, 2, 1, 3)
    ).reshape(GT, P, cmax * HDIM)
    pc_p = np.ascontiguousarray(
        pc.reshape(GT, cmax, P, HDIM).transpose(0, 2, 1, 3)
    ).reshape(GT, P, cmax * HDIM)
    # h-major (transposed), pre-chunked: [GT, P, KH*epc] (partition = h%128)
    pht = np.ascontiguousarray(phb.transpose(0, 2, 1))      # [GT, 256, epc]
    pht_p = np.ascontiguousarray(
        pht.reshape(GT, KH, P, epc).transpose(0, 2, 1, 3)
    ).reshape(GT, P, KH * epc)

    segrel = np.where(valid, seg[idxc] - P * np.arange(GT)[:, None],
                      -1).astype(np.int8)          # [GT, epc]
    segb = np.broadcast_to(segrel[:, None, :], (GT, P, epc))
    segc_p = np.full((GT, P, 8), -1, np.int8)
    segc_p[:, :, :cmax] = segrel.reshape(GT, cmax, P).transpose(0, 2, 1)

    # xT global, pre-chunked: [GT, P, KX*P]  (partition = x-row % 128)
    xt_g = np.zeros((XPAD, N), np.float32)
    xt_g[:EDIM] = x.T
    xt_g[EDIM] = 1.0
    xt_p = np.ascontiguousarray(
        xt_g.astype(bf16).reshape(KX, P, GT, P).transpose(2, 1, 0, 3)
    ).reshape(GT, P, KX * P)

    pk = np.concatenate([ph_p, pc_p, pht_p, xt_p], axis=2)  # bf16
    pk8i = np.concatenate([segb, segc_p], axis=2)    # [GT, P, 648] int8

    # padded weights; ones-row folds biases into the matmuls
    wf_pad = np.zeros((XPAD, HDIM), np.float32)
    wf_pad[:EDIM] = W_f
    wf_pad[EDIM] = b_f
    wc_pad = np.zeros((XPAD + HDIM, 3 * HDIM), np.float32)
    wc_pad[:EDIM] = W_combined[:EDIM]
    wc_pad[EDIM] = b_combined
    wc_pad[XPAD:] = W_combined[EDIM:]

    in_maps = []
    for c in range(NC):
        g0, g1 = c * NT, (c + 1) * NT
        in_maps.append({
            "pk": np.ascontiguousarray(pk[g0:g1]),
            "pk8i": np.ascontiguousarray(pk8i[g0:g1]),
            "wf": wf_pad.astype(bf16),
            "wc": wc_pad.astype(bf16),
            "uf": U_f.astype(bf16),
        })
    return in_maps, cmax


def _build(cmax, nt=NT, debug=None):
    import concourse.bass as bass
    import concourse.mybir as mybir
    import concourse.tile as tile
    from concourse import bacc

    dt = mybir.dt.float32
    bt = mybir.dt.bfloat16
    i8 = mybir.dt.int8
    epc = cmax * P
    H = HDIM
    # packed layout offsets (in elements) within pk[t, p, :]
    o_pc = cmax * H
    o_pht = o_pc + cmax * H
    o_xt = o_pht + KH * epc
    W = o_xt + KX * P

    nc = bacc.Bacc("TRN2", target_bir_lowering=False, debug=False,
                   num_devices=NC)
    pk_d = nc.declare_dram_parameter("pk", [NT, P, W], bt, isOutput=False)
    pk8i_d = nc.declare_dram_parameter("pk8i", [NT, P, epc + 8], i8,
                                       isOutput=False)
    wf_d = nc.declare_dram_parameter("wf", [XPAD, HDIM], bt, isOutput=False)
    wc_d = nc.declare_dram_parameter("wc", [XPAD + HDIM, 3 * HDIM], bt,
                                     isOutput=False)
    uf_d = nc.declare_dram_parameter("uf", [HDIM, HDIM], bt, isOutput=False)
    o_d = nc.declare_dram_parameter("out", [NT, P, 2 * HDIM], bt,
                                    isOutput=True)

    with tile.TileContext(nc) as tc:
        with (
            tc.tile_pool(name="const", bufs=1) as cpool,
            tc.tile_pool(name="inp", bufs=3) as ipool,
            tc.tile_pool(name="work", bufs=4) as wpool,
            tc.tile_pool(name="outp", bufs=3) as opool,
            tc.tile_pool(name="p_acc", bufs=1, space="PSUM") as p_acc,
            tc.tile_pool(name="p_fpre", bufs=1, space="PSUM") as p_fpre,
            tc.tile_pool(name="p_zio", bufs=2, space="PSUM") as p_zio,
            tc.tile_pool(name="p_fxb", bufs=1, space="PSUM") as p_fxb,
        ):
            # constants
            wf_sb = cpool.tile([P, KX, HDIM], bt)
            nc.sync.dma_start(out=wf_sb[:],
                              in_=wf_d.ap().rearrange("(k p) n -> p k n", p=P))
            wc_sb = cpool.tile([P, KALL, 3 * HDIM], bt)
            nc.sync.dma_start(out=wc_sb[:],
                              in_=wc_d.ap().rearrange("(k p) n -> p k n", p=P))
            uf_sb = cpool.tile([P, KH, HDIM], bt)
            nc.sync.dma_start(out=uf_sb[:],
                              in_=uf_d.ap().rearrange("(k p) n -> p k n", p=P))
            iota_i = cpool.tile([P, P], mybir.dt.int32)
            nc.gpsimd.iota(iota_i[:], pattern=[[1, P]], base=0,
                           channel_multiplier=0)
            iota_row5 = cpool.tile([P, cmax, P], i8)
            for s in range(cmax):
                nc.vector.tensor_copy(iota_row5[:, s, :], iota_i[:])
            iota_ci = cpool.tile([P, 1], mybir.dt.int32)
            nc.gpsimd.iota(iota_ci[:], pattern=[[1, 1]], base=0,
                           channel_multiplier=1)
            iota_col = cpool.tile([P, 1], i8)
            nc.vector.tensor_copy(iota_col[:], iota_ci[:])

            # persistent PSUM tiles (manual in-bank slot rotation)
            fpre6 = p_fpre.tile([P, 6, HDIM], dt, tag="fpre", bufs=1)
            zio2 = [p_zio.tile([P, 2 * HDIM], dt, name=f"zio{i}",
                               tag=f"zio{i}", bufs=1) for i in range(2)]
            # fxb group closes before the transposes run, so they can share
            # a bank; an open accumulation must never share a bank with a
            # start=True writer (the start clears the bank's has_written).
            fxtr = p_fxb.tile([P, 2 * HDIM], dt, tag="fxtr", bufs=1)
            fxb1 = fxtr[:, 0:HDIM]
            # acc double-buffers; zu(t) borrows the *other* acc buffer's
            # first half (its scatter won't start until well after tzu reads)
            acc2 = [p_acc.tile([P, 2 * HDIM], dt, name=f"acc{i}",
                               tag=f"acc{i}", bufs=1) for i in range(2)]

            def front(t):
                """loads + onehots + fxb + zio-x + fpre/sigmoid/fc of tile t"""
                big = ipool.tile([P, W], bt, tag="big", bufs=4)
                nc.sync.dma_start(out=big[:], in_=pk_d.ap()[t])
                seg8 = ipool.tile([P, epc + 8], i8, tag="seg8", bufs=4)
                nc.sync.dma_start(out=seg8[:], in_=pk8i_d.ap()[t])

                def phv(s, k, big=big):
                    b = s * H + k * P
                    return big[:, b:b + P]

                def pcv(s, big=big):
                    return big[:, o_pc + s * H:o_pc + (s + 1) * H]

                def phtv(k, s, big=big):
                    b = o_pht + k * epc + s * P
                    return big[:, b:b + P]

                def xtv(k, big=big):
                    b = o_xt + k * P
                    return big[:, b:b + P]

                zio = zio2[t % 2]
                fc_t = wpool.tile([P, cmax, HDIM], bt, tag="fc", bufs=2)

                ohT_all = wpool.tile([P, epc], bt, tag="ohT", bufs=2)
                nc.vector.tensor_tensor(
                    ohT_all[:], iota_col[:].to_broadcast([P, epc]),
                    seg8[:, 0:epc], op=mybir.AluOpType.is_equal)
                oh_all = wpool.tile([P, cmax, P], bt, tag="oh", bufs=2)
                nc.vector.tensor_tensor(
                    oh_all[:], seg8[:, epc:epc + cmax]
                    .to_broadcast([P, cmax, P]),
                    iota_row5[:], op=mybir.AluOpType.is_equal)

                for k in range(KX):
                    nc.tensor.matmul(fxb1, lhsT=xtv(k), rhs=wf_sb[:, k, :],
                                     start=(k == 0), stop=(k == KX - 1))
                fxb = wpool.tile([P, HDIM], bt, tag="fxb", bufs=2)
                nc.any.tensor_copy(fxb[:], fxb1)

                npair = (cmax + 1) // 2
                for g in range(npair):
                    s0 = 2 * g
                    ns = min(2, cmax - s0)
                    base = (g % 3) * 2
                    for i in range(ns):
                        s = s0 + i
                        fpre = fpre6[:, base + i, :]
                        nc.tensor.matmul(fpre,
                                         lhsT=ohT_all[:, s * P:(s + 1) * P],
                                         rhs=fxb[:], start=True, stop=False)
                        for k in range(KH):
                            nc.tensor.matmul(fpre, lhsT=phtv(k, s),
                                             rhs=uf_sb[:, k, :],
                                             start=False, stop=(k == KH - 1))
                    f_sb = wpool.tile([P, 2, HDIM], bt, tag="f", bufs=3)
                    nc.scalar.activation(
                        f_sb[:, 0:ns, :], fpre6[:, base:base + ns, :],
                        mybir.ActivationFunctionType.Sigmoid)
                    for i in range(ns):
                        s = s0 + i
                        nc.any.tensor_mul(fc_t[:, s, :], f_sb[:, i, :],
                                          pcv(s))

                for k in range(KX):
                    nc.tensor.matmul(zio[:], lhsT=xtv(k),
                                     rhs=wc_sb[:, k, 0:2 * HDIM],
                                     start=(k == 0), stop=False)

                return dict(t=t, fc_t=fc_t, oh_all=oh_all, zio=zio,
                            xtv=xtv, phv=phv, fxb=fxb)

            def back(st):
                """scatter + h_tildeT + z tails + gates + out of tile t"""
                t = st["t"]
                fc_t, oh_all, zio, xtv, phv = (
                    st["fc_t"], st["oh_all"], st["zio"], st["xtv"], st["phv"])
                # bank roles alternate: h_tildeT(t) in bank t%2; fc_acc(t)
                # and zu(t) in bank (t+1)%2 (their groups close before that
                # bank's next start=True clears it)
                htTps = acc2[t % 2][:, 0:HDIM]
                fcacc = acc2[(t + 1) % 2][:, HDIM:2 * HDIM]
                zu1 = acc2[(t + 1) % 2][:, 0:HDIM]

                # transposed h-scatter: h_tildeT[h, n] = sum_e ph[e,h]oh[e,n]
                # (single start=True: the bank-level has_written clear covers
                # both k-regions)
                for s in range(cmax):
                    for k in range(KH):
                        nc.tensor.matmul(htTps[:, k * P:(k + 1) * P],
                                         lhsT=phv(s, k),
                                         rhs=oh_all[:, s, :],
                                         start=(s == 0 and k == 0),
                                         stop=(s == cmax - 1 and k == KH - 1))
                for s in range(cmax):
                    nc.tensor.matmul(fcacc, lhsT=oh_all[:, s, :],
                                     rhs=fc_t[:, s, :],
                                     start=(s == 0), stop=(s == cmax - 1))
                htT = wpool.tile([P, KH * P], bt, tag="htT", bufs=2)
                nc.any.tensor_copy(htT[:], htTps)
                for k in range(KX):
                    nc.tensor.matmul(zu1, lhsT=xtv(k),
                                     rhs=wc_sb[:, k, 2 * HDIM:3 * HDIM],
                                     start=(k == 0), stop=False)

                for k in range(KH):
                    kk = KX + k
                    nc.tensor.matmul(zio[:], lhsT=htT[:, k * P:(k + 1) * P],
                                     rhs=wc_sb[:, kk, 0:2 * HDIM],
                                     start=False, stop=(k == KH - 1))
                    nc.tensor.matmul(zu1, lhsT=htT[:, k * P:(k + 1) * P],
                                     rhs=wc_sb[:, kk, 2 * HDIM:3 * HDIM],
                                     start=False, stop=(k == KH - 1))

                if debug == "acc":
                    out_t = opool.tile([P, 2 * HDIM], bt)
                    nc.vector.tensor_copy(out_t[:, 0:HDIM], htTps)
                    nc.vector.tensor_copy(out_t[:, HDIM:2 * HDIM], fcacc)
                    nc.sync.dma_start(out=o_d.ap()[t], in_=out_t[:])
                    return
                if debug == "z":
                    out_t = opool.tile([P, 2 * HDIM], bt)
                    nc.vector.tensor_copy(out_t[:, 0:HDIM], zio[:, 0:HDIM])
                    nc.vector.tensor_copy(out_t[:, HDIM:2 * HDIM], zu1)
                    nc.sync.dma_start(out=o_d.ap()[t], in_=out_t[:])
                    return

                out_t = opool.tile([P, 2 * HDIM], bt)
                szio = wpool.tile([P, 2 * HDIM], dt, tag="szio", bufs=2)
                nc.scalar.activation(szio[:], zio[:],
                                     mybir.ActivationFunctionType.Sigmoid)
                tzu = wpool.tile([P, HDIM], dt, tag="tzu", bufs=2)
                nc.scalar.activation(tzu[:], zu1,
                                     mybir.ActivationFunctionType.Tanh)
                ci = wpool.tile([P, HDIM], dt, tag="ci", bufs=2)
                nc.any.tensor_mul(ci[:], szio[:, 0:HDIM], tzu[:])
                nc.any.tensor_add(out_t[:, 0:HDIM], ci[:], fcacc)
                tc_sb = wpool.tile([P, HDIM], dt, tag="tc", bufs=2)
                nc.scalar.activation(tc_sb[:], out_t[:, 0:HDIM],
                                     mybir.ActivationFunctionType.Tanh)
                nc.any.tensor_mul(out_t[:, HDIM:2 * HDIM],
                                  szio[:, HDIM:2 * HDIM], tc_sb[:])
                nc.sync.dma_start(out=o_d.ap()[t], in_=out_t[:])

            pend = None
            for t in range(nt):
                st = front(t)
                if pend is not None:
                    back(pend)
                pend = st
            back(pend)

    nc.compile()
    return nc


def kernel(x, prev_c, prev_h, W_combined, b_combined, W_f, U_f, b_f,
           segment_ids, _trace=False, _debug=None):
    from concourse.bass_utils import run_bass_kernel_spmd

    in_maps, cmax = _preprocess(x, prev_c, prev_h, W_combined, b_combined,
                                W_f, U_f, b_f, segment_ids)
    nc = _build(cmax, debug=_debug)
    res = run_bass_kernel_spmd(nc, in_maps, list(range(NC)), trace=_trace)
    cs, hs = [], []
    for i in range(NC):
        o = np.asarray(res.results[i]["out"]).astype(np.float32)
        o = o.reshape(NLOC, 2 * HDIM)
        cs.append(o[:, :HDIM])
        hs.append(o[:, HDIM:])
    c = np.concatenate(cs, axis=0)
    h = np.concatenate(hs, axis=0)
    kernel._last_exec_time_ns = res.exec_time_ns
    return (c, h)
